# revision 38
# baseline (speedup 1.0000x reference)
"""Trainium2 Bass kernel for MultiHeadAttention (B=2, S=2048, D=1024, H=16).

Sharding: 8 cores = 2 (batch) x 4 (head groups of 4 heads / 256 proj cols).
Each core computes attention for its batch + head group and a partial
output projection [S, D]; host sums the 4 partials per batch and adds bo.

Device pipeline per core (all matmuls in float32r = fp22, full PE rate):
  1. Project from host-pretransposed activations/weights:
       K.T[o,s], Q.T[o,s]  (lhsT = W.T, rhs = x.T)
       V[s,o]              (lhsT = x.T, rhs = W.T), ones-augmented per head
  2. Per sq-chunk c (512), per sk-tile j, per head h:
     S.T[sk,sq] = K.T_h^T Q.T_h (K=64; head pairs land on PE row groups
     0-63/64-127 so two heads run concurrently), additive -3e4 mask bias
     on partial blocks (block structure from the real mask, computed on
     host), exp (scale=1/8) -> P.T, PV (K=128) -> Z.T_aug (row 64 =
     softmax denominator); then reciprocal + K=1 matmul broadcast ->
     scale Z.T into SBUF.
  3. Out-proj per s-tile: O_partial[s, dout] = Z.T^T @ Wo_g.T, DMA out.
"""

import math
import os
import sys

import numpy as np

sys.path.insert(0, "/opt/trn_rl_repo")
sys.path.insert(0, "/opt/trn_rl_repo/concourse")

B, S, D, H = 2, 2048, 1024, 16
HD = D // H  # 64
G = 4  # head groups (cores per batch)
OG = D // G  # 256 proj cols per core
HPG = H // G  # 4 heads per core
P = 128
NT = S // P  # 16 s-tiles
CH = 512  # sq chunk width
NCH = S // CH  # 4 chunks
KT = D // P  # 8 contraction tiles for projections
NEG = -30000.0  # additive mask bias (pre-scale)

_cache = {}


def _block_structure(mask, key_padding_mask):
    """Classify each 128x128 block of the [S,S] score matrix per batch.

    Returns (process, biased, bias_data) where
      process[i,j]  : bool  -- any batch needs block (sq-tile i, sk-tile j)
      biased[i,j]   : bool  -- some processed batch needs a bias on (i,j)
      bias_data[b]  : {(i,j): [128,128] f32 bias (TRANSPOSED: [sk,sq])}
    """
    mask = np.asarray(mask)
    kpm = np.asarray(key_padding_mask)
    full = np.zeros((B, NT, NT), dtype=bool)
    partial = np.zeros((B, NT, NT), dtype=bool)
    blocks = {}
    for b in range(B):
        for i in range(NT):
            mrow = mask[i * P:(i + 1) * P]
            for j in range(NT):
                mb = mrow[:, j * P:(j + 1) * P] | kpm[b, None, j * P:(j + 1) * P]
                if mb.all():
                    full[b, i, j] = True
                elif mb.any():
                    partial[b, i, j] = True
                    blocks[(b, i, j)] = mb
                else:
                    blocks[(b, i, j)] = None
    process = (~full).any(axis=0)
    biased = process & (full | partial).any(axis=0)
    bias_data = []
    for b in range(B):
        d = {}
        for i in range(NT):
            for j in range(NT):
                if not (process[i, j] and biased[i, j]):
                    continue
                if full[b, i, j]:
                    d[(i, j)] = np.full((P, P), NEG, np.float32)
                elif partial[b, i, j]:
                    d[(i, j)] = (blocks[(b, i, j)].T * NEG).astype(np.float32)
                else:
                    d[(i, j)] = np.zeros((P, P), np.float32)
        bias_data.append(d)
    return process, biased, bias_data


def _build_bass(process, biased, bias_slots):
    """Trace the Tile kernel. bias_slots: {(i,j): slot} for biased blocks."""
    import concourse.bass as bass
    import concourse.tile as tile
    from concourse import bacc, mybir

    f32 = mybir.dt.float32
    f32r = mybir.dt.float32r
    bf16 = mybir.dt.bfloat16
    nc = bacc.Bacc("TRN2", target_bir_lowering=False, debug=False,
                   enable_asserts=False)

    # Host supplies activations and weights already transposed.
    xqT = nc.dram_tensor("xqT", [D, S], bf16, kind="ExternalInput").ap()
    xkT = nc.dram_tensor("xkT", [D, S], bf16, kind="ExternalInput").ap()
    xvT = nc.dram_tensor("xvT", [D, S], bf16, kind="ExternalInput").ap()
    wqT = nc.dram_tensor("wqT", [D, OG], bf16, kind="ExternalInput").ap()
    wkT = nc.dram_tensor("wkT", [D, OG], bf16, kind="ExternalInput").ap()
    wvT = nc.dram_tensor("wvT", [D, OG], bf16, kind="ExternalInput").ap()
    woT = nc.dram_tensor("woT", [OG, D], f32r, kind="ExternalInput").ap()
    bq = nc.dram_tensor("bq", [OG], f32, kind="ExternalInput").ap()
    bk = nc.dram_tensor("bk", [OG], f32, kind="ExternalInput").ap()
    bv = nc.dram_tensor("bv", [OG], f32, kind="ExternalInput").ap()
    nbias = max(1, len(bias_slots))
    biasT = nc.dram_tensor("biasT", [nbias, P, P], f32,
                           kind="ExternalInput").ap()
    out = nc.dram_tensor("out", [S, D], bf16, kind="ExternalOutput").ap()

    xqTr = xqT.rearrange("(t p) s -> p t s", p=P)
    xkTr = xkT.rearrange("(t p) s -> p t s", p=P)
    xvTr = xvT.rearrange("(t p) s -> p t s", p=P)

    with tile.TileContext(nc) as tc:
        with tc.tile_pool(name="persist", bufs=1) as persist, \
             tc.tile_pool(name="const", bufs=1) as const:
            # Persistent SBUF tensors
            qT = persist.tile([P, 2, S], f32r)       # [o-part, o-tile, s]
            kT = persist.tile([P, 2, S], f32r)
            vaug = persist.tile([P, NT, HPG, HD + 1], f32r)
            zt01 = persist.tile([P, S], f32r)        # heads 0,1 Z.T scaled
            zt23 = persist.tile([P, S], f32r)
            woT_sb = persist.tile([P, 2, D], f32r)
            bias_sb = persist.tile([P, nbias, P], f32)

            ones_row = const.tile([1, P], f32r)
            one_bits = 0x3F800000  # 1.0f
            nc.vector.memset(ones_row.bitcast(mybir.dt.uint32), one_bits)
            bqs = const.tile([P, 2], f32)
            bks = const.tile([P, 2], f32)
            bvb = const.tile([P, OG], f32)

            nc.sync.dma_start(bqs, bq.rearrange("(t p) -> p t", p=P))
            nc.sync.dma_start(bks, bk.rearrange("(t p) -> p t", p=P))
            # broadcast bv across partitions
            nc.sync.dma_start(
                bvb, bass.AP(tensor=bv.tensor, offset=bv.offset,
                             ap=[[0, P]] + list(bv.ap)))
            nc.vector.memset(vaug[:, :, :, HD:HD + 1].bitcast(mybir.dt.uint32),
                             one_bits)

            # ---- Flat pools for the whole kernel (avoid release stalls) ----
            xTp = tc.alloc_tile_pool(name="xT", bufs=3)
            wsb = tc.alloc_tile_pool(name="wsb", bufs=1)
            psum = tc.alloc_tile_pool(name="psum", bufs=1, space="PSUM")
            ptp = tc.alloc_tile_pool(name="pt", bufs=6)
            small = tc.alloc_tile_pool(name="small", bufs=4)
            osb = tc.alloc_tile_pool(name="osb", bufs=3)
            if True:
                wqT_sb = wsb.tile([P, KT, OG], bf16, tag="w")
                wkT_sb = wsb.tile([P, KT, OG], bf16, tag="w2")
                wvT_sb = wsb.tile([P, KT, OG], bf16, tag="w3")
                nc.sync.dma_start(wkT_sb, wkT.rearrange("(t p) o -> p t o", p=P))
                nc.sync.dma_start(wvT_sb, wvT.rearrange("(t p) o -> p t o", p=P))
                nc.sync.dma_start(wqT_sb, wqT.rearrange("(t p) o -> p t o", p=P))

                # Projections ordered so attention can start ASAP:
                # all K chunks, Q chunk 0, all V chunks, Q chunks 1-3.
                # Deferred constant loads (bias tiles, Wo) are emitted
                # mid-stream so they don't delay the first projections.
                plan = ([(0, c) for c in range(NCH)] + [(2, 0)]
                        + [(1, c) for c in range(NCH)]
                        + [(2, c) for c in range(1, NCH)])
                srcs = {0: (xkTr, wkT_sb), 1: (xvTr, wvT_sb),
                        2: (xqTr, wqT_sb)}
                for step, (which, c) in enumerate(plan):
                    if step == 5:
                        nc.sync.dma_start(bias_sb,
                                          biasT.rearrange("n p q -> p n q"))
                    elif step == 8:
                        nc.sync.dma_start(
                            woT_sb, woT.rearrange("(t p) d -> p t d", p=P))
                    if True:
                        xr, w_sb = srcs[which]
                        xTc = xTp.tile([P, KT, CH], bf16, tag="xT",
                                       name="xTc")
                        nc.sync.dma_start(xTc, xr[:, :, c * CH:(c + 1) * CH])
                        if which != 1:
                            # K.T / Q.T : out [o(2 tiles), s-chunk]
                            dst = kT if which == 0 else qT
                            bias_ap = bks if which == 0 else bqs
                            for ot in range(2):
                                ps = psum.tile([P, CH], f32, tag="ps512",
                                               bufs=4, name="ps")
                                for k in range(KT):
                                    nc.tensor.matmul(
                                        ps, w_sb[:, k, ot * P:(ot + 1) * P],
                                        xTc[:, k, :],
                                        start=(k == 0), stop=(k == KT - 1))
                                nc.vector.tensor_scalar_add(
                                    dst[:, ot, c * CH:(c + 1) * CH], ps,
                                    bias_ap[:, ot:ot + 1])
                        else:
                            # V: out [s-tile, o]; bias broadcast via DVE
                            for st in range(CH // P):
                                ps = psum.tile([P, OG], f32, tag="ps512",
                                               bufs=4, name="ps")
                                for k in range(KT):
                                    nc.tensor.matmul(
                                        ps, xTc[:, k, st * P:(st + 1) * P],
                                        w_sb[:, k, :],
                                        start=(k == 0), stop=(k == KT - 1))
                                nc.vector.tensor_add(
                                    vaug[:, c * 4 + st, :, 0:HD],
                                    ps.rearrange("p (h d) -> p h d", h=HPG),
                                    bvb.rearrange("p (h d) -> p h d", h=HPG))

            # ---- Attention + out-proj, per sq-chunk ----
            if True:
                for c in range(NCH):
                    tiles_i = list(range(c * 4, c * 4 + 4))
                    jplan = []
                    for j in range(NT):
                        ii = [i for i in tiles_i if process[i, j]]
                        if ii:
                            jplan.append((j, min(ii) - c * 4,
                                          max(ii) - c * 4 + 1))
                    for hp in range(2):  # head pairs (2*hp, 2*hp+1)
                        h0, h1 = 2 * hp, 2 * hp + 1
                        ot = hp
                        ztaus = {}
                        for h in (h0, h1):
                            zta = psum.tile([HD + 1, CH], f32,
                                            tag=f"zt{h % 2}", bufs=1,
                                            name=f"ztau{h % 2}")
                            ztaus[h] = zta
                        first = True
                        for j, lo, hi in jplan:
                            off, w = lo * P, (hi - lo) * P
                            # both heads' S.T in one [P, 2*CH] psum tile:
                            # h0 -> cols [0, CH), h1 -> cols [CH, 2CH);
                            # base partitions 0/64 put them on different
                            # PE row groups (concurrent matmuls).
                            st_ = psum.tile([P, 2 * CH], f32, tag="st",
                                            bufs=2, name="st_")
                            for hh, h in enumerate((h0, h1)):
                                po = (h % 2) * HD
                                nc.tensor.matmul(
                                    st_[:, hh * CH + off:hh * CH + off + w],
                                    kT[po:po + HD, ot, j * P:(j + 1) * P],
                                    qT[po:po + HD, ot,
                                       c * CH + off:c * CH + off + w],
                                    start=True, stop=True)
                            for i in range(c * 4 + lo, c * 4 + hi):
                                if biased[i, j]:
                                    sl = bias_slots[(i, j)]
                                    so = (i - c * 4) * P
                                    bap = bias_sb[:, sl, :]
                                    bcast2 = bass.AP(
                                        tensor=bap.tensor, offset=bap.offset,
                                        ap=[bap.ap[0], [0, 2]] + list(bap.ap[1:]))
                                    stv = st_[:, so:so + P]
                                    st2 = bass.AP(
                                        tensor=stv.tensor, offset=stv.offset,
                                        ap=[stv.ap[0], [CH, 2]] + list(stv.ap[1:]))
                                    nc.vector.tensor_add(st2, st2, bcast2)
                            pt = ptp.tile([P, 2 * CH], f32r, tag="pt",
                                          name="pt")
                            pt2 = pt.rearrange("p (a b) -> p a b", a=2)
                            stq = st_.rearrange("p (a b) -> p a b", a=2)
                            nc.scalar.activation(
                                pt2[:, :, off:off + w], stq[:, :, off:off + w],
                                mybir.ActivationFunctionType.Exp,
                                scale=1.0 / math.sqrt(HD))
                            for hh, h in enumerate((h0, h1)):
                                nc.tensor.matmul(
                                    ztaus[h][:, off:off + w],
                                    vaug[:, j, h, :],
                                    pt[:, hh * CH + off:hh * CH + off + w],
                                    start=first, stop=(j == jplan[-1][0]))
                            first = False
                        for h in (h0, h1):
                            zdst = zt01 if h < 2 else zt23
                            zpo = (h % 2) * HD
                            recip = small.tile([1, CH], f32r, tag="recip",
                                               name="recip")
                            with nc.allow_low_precision(reason="fp22 recip"):
                                nc.vector.reciprocal(recip,
                                                     ztaus[h][HD:HD + 1, :])
                            bc = psum.tile([P, CH], f32, tag="ps512", bufs=4,
                                           name="bc")
                            nc.tensor.matmul(bc, ones_row, recip,
                                             start=True, stop=True)
                            bcs = small.tile([P, CH], f32, tag="bcs",
                                             name="bcs")
                            if h % 2 == 0:
                                nc.scalar.copy(bcs, bc)
                            else:
                                nc.vector.tensor_copy(bcs, bc)
                            nc.vector.tensor_mul(
                                zdst[zpo:zpo + HD, c * CH:(c + 1) * CH],
                                ztaus[h][0:HD, :], bcs[0:HD, :])
                    # out-proj for this chunk's 4 s-tiles
                    for st in range(4):
                        sg = c * 4 + st
                        ob = osb.tile([P, D], bf16, tag="ob", name="ob")
                        for nchunk in range(2):
                            ps = psum.tile([P, CH], f32, tag="ps512",
                                           bufs=4, name="ps")
                            for k, zsrc in enumerate((zt01, zt23)):
                                nc.tensor.matmul(
                                    ps, zsrc[:, sg * P:(sg + 1) * P],
                                    woT_sb[:, k, nchunk * CH:(nchunk + 1) * CH],
                                    start=(k == 0), stop=(k == 1))
                            if nchunk == 0:
                                nc.scalar.copy(
                                    ob[:, nchunk * CH:(nchunk + 1) * CH], ps)
                            else:
                                nc.vector.tensor_copy(
                                    ob[:, nchunk * CH:(nchunk + 1) * CH], ps)
                        nc.sync.dma_start(out[sg * P:(sg + 1) * P, :], ob)
            for pool_ in (osb, small, ptp, psum, wsb, xTp):
                pool_.release()
    nc.compile()
    # Belt-and-braces: any write-only preamble registers that survive DCE
    # but never get ids from alloc_regs would fail walrus birverifier
    # (reg_id == -1). They are write-only, so engine-unique ids are safe;
    # keep _lo/_hi pairs adjacent and even-aligned.
    from collections import defaultdict
    ctr = defaultdict(int)
    for f_ in nc.m.functions:
        for a in f_.allocations:
            if isinstance(a, mybir.Register) and a.reg_id >= 0:
                ctr[a.engine] = max(ctr[a.engine], a.reg_id + 1)
    for f_ in nc.m.functions:
        for a in f_.allocations:
            if isinstance(a, mybir.Register) and a.reg_id == -1:
                if a.name.endswith("_lo") and ctr[a.engine] % 2:
                    ctr[a.engine] += 1
                a.reg_id = ctr[a.engine]
                ctr[a.engine] += 1
    return nc


def kernel(query, key, value, mask, key_padding_mask,
           Wq, bq, Wk, bk, Wv, bv, Wo, bo, _return_perf=False):
    from concourse import bass_utils

    query = np.asarray(query, np.float32)
    key_ = np.asarray(key, np.float32)
    value = np.asarray(value, np.float32)
    Wq, Wk, Wv, Wo = (np.asarray(w, np.float32) for w in (Wq, Wk, Wv, Wo))
    bq, bk, bv, bo = (np.asarray(b_, np.float32) for b_ in (bq, bk, bv, bo))

    process, biased, bias_data = _block_structure(mask, key_padding_mask)
    bias_slots = {}
    for i in range(NT):
        for j in range(NT):
            if process[i, j] and biased[i, j]:
                bias_slots[(i, j)] = len(bias_slots)

    key_struct = (process.tobytes(), biased.tobytes())
    if key_struct not in _cache:
        _cache[key_struct] = _build_bass(process, biased, bias_slots)
    nc = _cache[key_struct]

    nbias = max(1, len(bias_slots))
    import ml_dtypes
    bf = ml_dtypes.bfloat16
    xT = {}
    for b in range(B):
        xT[("q", b)] = np.ascontiguousarray(query[b].T.astype(bf))
        xT[("k", b)] = np.ascontiguousarray(key_[b].T.astype(bf))
        xT[("v", b)] = np.ascontiguousarray(value[b].T.astype(bf))
    in_maps = []
    for core in range(8):
        b, g = core // G, core % G
        sl = slice(g * OG, (g + 1) * OG)
        bt = np.zeros((nbias, P, P), np.float32)
        for (i, j), slot in bias_slots.items():
            bt[slot] = bias_data[b][(i, j)]
        in_maps.append({
            "xqT": xT[("q", b)],
            "xkT": xT[("k", b)],
            "xvT": xT[("v", b)],
            "wqT": np.ascontiguousarray(Wq[sl].T.astype(bf)),
            "wkT": np.ascontiguousarray(Wk[sl].T.astype(bf)),
            "wvT": np.ascontiguousarray(Wv[sl].T.astype(bf)),
            "woT": np.ascontiguousarray(Wo[:, sl].T),
            "bq": np.ascontiguousarray(bq[sl]),
            "bk": np.ascontiguousarray(bk[sl]),
            "bv": np.ascontiguousarray(bv[sl]),
            "biasT": bt,
        })

    trace = bool(int(os.environ.get("KERNEL_TRACE", "0")))
    res = bass_utils.run_bass_kernel_spmd(
        nc, in_maps, core_ids=list(range(8)), trace=trace)

    out = np.zeros((B, S, D), np.float32)
    for core in range(8):
        out[core // G] += res.results[core]["out"].astype(np.float32)
    out += bo[None, None, :]
    if _return_perf:
        return out, res
    return out


# revision 39
# speedup vs baseline: 1.0012x; 1.0012x over previous
"""Trainium2 Bass kernel for MultiHeadAttention (B=2, S=2048, D=1024, H=16).

Sharding: 8 cores = 2 (batch) x 4 (head groups of 4 heads / 256 proj cols).
Each core computes attention for its batch + head group and a partial
output projection [S, D]; host sums the 4 partials per batch and adds bo.

Device pipeline per core (all matmuls in float32r = fp22, full PE rate):
  1. Project from host-pretransposed activations/weights:
       K.T[o,s], Q.T[o,s]  (lhsT = W.T, rhs = x.T)
       V[s,o]              (lhsT = x.T, rhs = W.T), ones-augmented per head
  2. Per sq-chunk c (512), per sk-tile j, per head h:
     S.T[sk,sq] = K.T_h^T Q.T_h (K=64; head pairs land on PE row groups
     0-63/64-127 so two heads run concurrently), additive -3e4 mask bias
     on partial blocks (block structure from the real mask, computed on
     host), exp (scale=1/8) -> P.T, PV (K=128) -> Z.T_aug (row 64 =
     softmax denominator); then reciprocal + K=1 matmul broadcast ->
     scale Z.T into SBUF.
  3. Out-proj per s-tile: O_partial[s, dout] = Z.T^T @ Wo_g.T, DMA out.
"""

import math
import os
import sys

import numpy as np

sys.path.insert(0, "/opt/trn_rl_repo")
sys.path.insert(0, "/opt/trn_rl_repo/concourse")

B, S, D, H = 2, 2048, 1024, 16
HD = D // H  # 64
G = 4  # head groups (cores per batch)
OG = D // G  # 256 proj cols per core
HPG = H // G  # 4 heads per core
P = 128
NT = S // P  # 16 s-tiles
CH = 512  # sq chunk width
NCH = S // CH  # 4 chunks
KT = D // P  # 8 contraction tiles for projections
NEG = -30000.0  # additive mask bias (pre-scale)

_cache = {}


def _block_structure(mask, key_padding_mask):
    """Classify each 128x128 block of the [S,S] score matrix per batch.

    Returns (process, biased, bias_data) where
      process[i,j]  : bool  -- any batch needs block (sq-tile i, sk-tile j)
      biased[i,j]   : bool  -- some processed batch needs a bias on (i,j)
      bias_data[b]  : {(i,j): [128,128] f32 bias (TRANSPOSED: [sk,sq])}
    """
    mask = np.asarray(mask)
    kpm = np.asarray(key_padding_mask)
    full = np.zeros((B, NT, NT), dtype=bool)
    partial = np.zeros((B, NT, NT), dtype=bool)
    blocks = {}
    for b in range(B):
        for i in range(NT):
            mrow = mask[i * P:(i + 1) * P]
            for j in range(NT):
                mb = mrow[:, j * P:(j + 1) * P] | kpm[b, None, j * P:(j + 1) * P]
                if mb.all():
                    full[b, i, j] = True
                elif mb.any():
                    partial[b, i, j] = True
                    blocks[(b, i, j)] = mb
                else:
                    blocks[(b, i, j)] = None
    process = (~full).any(axis=0)
    biased = process & (full | partial).any(axis=0)
    bias_data = []
    for b in range(B):
        d = {}
        for i in range(NT):
            for j in range(NT):
                if not (process[i, j] and biased[i, j]):
                    continue
                if full[b, i, j]:
                    d[(i, j)] = np.full((P, P), NEG, np.float32)
                elif partial[b, i, j]:
                    d[(i, j)] = (blocks[(b, i, j)].T * NEG).astype(np.float32)
                else:
                    d[(i, j)] = np.zeros((P, P), np.float32)
        bias_data.append(d)
    return process, biased, bias_data


def _build_bass(process, biased, bias_slots):
    """Trace the Tile kernel. bias_slots: {(i,j): slot} for biased blocks."""
    import concourse.bass as bass
    import concourse.tile as tile
    from concourse import bacc, mybir

    f32 = mybir.dt.float32
    f32r = mybir.dt.float32r
    bf16 = mybir.dt.bfloat16
    nc = bacc.Bacc("TRN2", target_bir_lowering=False, debug=False,
                   enable_asserts=False)

    # Host supplies activations and weights already transposed.
    xqT = nc.dram_tensor("xqT", [D, S], bf16, kind="ExternalInput").ap()
    xkT = nc.dram_tensor("xkT", [D, S], bf16, kind="ExternalInput").ap()
    xvT = nc.dram_tensor("xvT", [D, S], bf16, kind="ExternalInput").ap()
    wqT = nc.dram_tensor("wqT", [D, OG], bf16, kind="ExternalInput").ap()
    wkT = nc.dram_tensor("wkT", [D, OG], bf16, kind="ExternalInput").ap()
    wvT = nc.dram_tensor("wvT", [D, OG], bf16, kind="ExternalInput").ap()
    woT = nc.dram_tensor("woT", [OG, D], f32r, kind="ExternalInput").ap()
    bq = nc.dram_tensor("bq", [OG], f32, kind="ExternalInput").ap()
    bk = nc.dram_tensor("bk", [OG], f32, kind="ExternalInput").ap()
    bv = nc.dram_tensor("bv", [OG], f32, kind="ExternalInput").ap()
    nbias = max(1, len(bias_slots))
    biasT = nc.dram_tensor("biasT", [nbias, P, P], f32,
                           kind="ExternalInput").ap()
    out = nc.dram_tensor("out", [S, D], bf16, kind="ExternalOutput").ap()

    xqTr = xqT.rearrange("(t p) s -> p t s", p=P)
    xkTr = xkT.rearrange("(t p) s -> p t s", p=P)
    xvTr = xvT.rearrange("(t p) s -> p t s", p=P)

    with tile.TileContext(nc) as tc:
        with tc.tile_pool(name="persist", bufs=1) as persist, \
             tc.tile_pool(name="const", bufs=1) as const:
            # Persistent SBUF tensors
            qT = persist.tile([P, 2, S], f32r)       # [o-part, o-tile, s]
            kT = persist.tile([P, 2, S], f32r)
            vaug = persist.tile([P, NT, HPG, HD + 1], bf16)
            zt01 = persist.tile([P, S], f32r)        # heads 0,1 Z.T scaled
            zt23 = persist.tile([P, S], f32r)
            woT_sb = persist.tile([P, 2, D], f32r)
            bias_sb = persist.tile([P, nbias, P], f32)

            ones_row = const.tile([1, P], f32r)
            one_bits = 0x3F800000  # 1.0f
            nc.vector.memset(ones_row.bitcast(mybir.dt.uint32), one_bits)
            bqs = const.tile([P, 2], f32)
            bks = const.tile([P, 2], f32)
            bvb = const.tile([P, OG], f32)

            nc.sync.dma_start(bqs, bq.rearrange("(t p) -> p t", p=P))
            nc.sync.dma_start(bks, bk.rearrange("(t p) -> p t", p=P))
            # broadcast bv across partitions
            nc.sync.dma_start(
                bvb, bass.AP(tensor=bv.tensor, offset=bv.offset,
                             ap=[[0, P]] + list(bv.ap)))
            nc.vector.memset(vaug[:, :, :, HD:HD + 1].bitcast(mybir.dt.uint16),
                             0x3F80)  # 1.0 in bf16

            # ---- Flat pools for the whole kernel (avoid release stalls) ----
            xTp = tc.alloc_tile_pool(name="xT", bufs=3)
            wsb = tc.alloc_tile_pool(name="wsb", bufs=1)
            psum = tc.alloc_tile_pool(name="psum", bufs=1, space="PSUM")
            ptp = tc.alloc_tile_pool(name="pt", bufs=6)
            small = tc.alloc_tile_pool(name="small", bufs=4)
            osb = tc.alloc_tile_pool(name="osb", bufs=3)
            if True:
                wqT_sb = wsb.tile([P, KT, OG], bf16, tag="w")
                wkT_sb = wsb.tile([P, KT, OG], bf16, tag="w2")
                wvT_sb = wsb.tile([P, KT, OG], bf16, tag="w3")
                nc.sync.dma_start(wkT_sb, wkT.rearrange("(t p) o -> p t o", p=P))
                nc.sync.dma_start(wvT_sb, wvT.rearrange("(t p) o -> p t o", p=P))
                nc.sync.dma_start(wqT_sb, wqT.rearrange("(t p) o -> p t o", p=P))

                # Projections ordered so attention can start ASAP:
                # all K chunks, Q chunk 0, all V chunks, Q chunks 1-3.
                # Deferred constant loads (bias tiles, Wo) are emitted
                # mid-stream so they don't delay the first projections.
                plan = ([(0, c) for c in range(NCH)] + [(2, 0)]
                        + [(1, c) for c in range(NCH)]
                        + [(2, c) for c in range(1, NCH)])
                srcs = {0: (xkTr, wkT_sb), 1: (xvTr, wvT_sb),
                        2: (xqTr, wqT_sb)}
                for step, (which, c) in enumerate(plan):
                    if step == 5:
                        nc.sync.dma_start(bias_sb,
                                          biasT.rearrange("n p q -> p n q"))
                    elif step == 8:
                        nc.sync.dma_start(
                            woT_sb, woT.rearrange("(t p) d -> p t d", p=P))
                    if True:
                        xr, w_sb = srcs[which]
                        xTc = xTp.tile([P, KT, CH], bf16, tag="xT",
                                       name="xTc")
                        nc.sync.dma_start(xTc, xr[:, :, c * CH:(c + 1) * CH])
                        if which != 1:
                            # K.T / Q.T : out [o(2 tiles), s-chunk]
                            dst = kT if which == 0 else qT
                            bias_ap = bks if which == 0 else bqs
                            for ot in range(2):
                                ps = psum.tile([P, CH], f32, tag="ps512",
                                               bufs=4, name="ps")
                                for k in range(KT):
                                    nc.tensor.matmul(
                                        ps, w_sb[:, k, ot * P:(ot + 1) * P],
                                        xTc[:, k, :],
                                        start=(k == 0), stop=(k == KT - 1))
                                nc.vector.tensor_scalar_add(
                                    dst[:, ot, c * CH:(c + 1) * CH], ps,
                                    bias_ap[:, ot:ot + 1])
                        else:
                            # V: out [s-tile, o]; bias broadcast via DVE
                            for st in range(CH // P):
                                ps = psum.tile([P, OG], f32, tag="ps512",
                                               bufs=4, name="ps")
                                for k in range(KT):
                                    nc.tensor.matmul(
                                        ps, xTc[:, k, st * P:(st + 1) * P],
                                        w_sb[:, k, :],
                                        start=(k == 0), stop=(k == KT - 1))
                                nc.vector.tensor_add(
                                    vaug[:, c * 4 + st, :, 0:HD],
                                    ps.rearrange("p (h d) -> p h d", h=HPG),
                                    bvb.rearrange("p (h d) -> p h d", h=HPG))

            # ---- Attention + out-proj, per sq-chunk ----
            if True:
                for c in range(NCH):
                    tiles_i = list(range(c * 4, c * 4 + 4))
                    jplan = []
                    for j in range(NT):
                        ii = [i for i in tiles_i if process[i, j]]
                        if ii:
                            jplan.append((j, min(ii) - c * 4,
                                          max(ii) - c * 4 + 1))
                    for hp in range(2):  # head pairs (2*hp, 2*hp+1)
                        h0, h1 = 2 * hp, 2 * hp + 1
                        ot = hp
                        ztaus = {}
                        for h in (h0, h1):
                            zta = psum.tile([HD + 1, CH], f32,
                                            tag=f"zt{h % 2}", bufs=1,
                                            name=f"ztau{h % 2}")
                            ztaus[h] = zta
                        first = True
                        for j, lo, hi in jplan:
                            off, w = lo * P, (hi - lo) * P
                            # both heads' S.T in one [P, 2*CH] psum tile:
                            # h0 -> cols [0, CH), h1 -> cols [CH, 2CH);
                            # base partitions 0/64 put them on different
                            # PE row groups (concurrent matmuls).
                            st_ = psum.tile([P, 2 * CH], f32, tag="st",
                                            bufs=2, name="st_")
                            for hh, h in enumerate((h0, h1)):
                                po = (h % 2) * HD
                                nc.tensor.matmul(
                                    st_[:, hh * CH + off:hh * CH + off + w],
                                    kT[po:po + HD, ot, j * P:(j + 1) * P],
                                    qT[po:po + HD, ot,
                                       c * CH + off:c * CH + off + w],
                                    start=True, stop=True)
                            for i in range(c * 4 + lo, c * 4 + hi):
                                if biased[i, j]:
                                    sl = bias_slots[(i, j)]
                                    so = (i - c * 4) * P
                                    bap = bias_sb[:, sl, :]
                                    bcast2 = bass.AP(
                                        tensor=bap.tensor, offset=bap.offset,
                                        ap=[bap.ap[0], [0, 2]] + list(bap.ap[1:]))
                                    stv = st_[:, so:so + P]
                                    st2 = bass.AP(
                                        tensor=stv.tensor, offset=stv.offset,
                                        ap=[stv.ap[0], [CH, 2]] + list(stv.ap[1:]))
                                    nc.vector.tensor_add(st2, st2, bcast2)
                            pt = ptp.tile([P, 2 * CH], bf16, tag="pt",
                                          name="pt")
                            pt2 = pt.rearrange("p (a b) -> p a b", a=2)
                            stq = st_.rearrange("p (a b) -> p a b", a=2)
                            nc.scalar.activation(
                                pt2[:, :, off:off + w], stq[:, :, off:off + w],
                                mybir.ActivationFunctionType.Exp,
                                scale=1.0 / math.sqrt(HD))
                            for hh, h in enumerate((h0, h1)):
                                nc.tensor.matmul(
                                    ztaus[h][:, off:off + w],
                                    vaug[:, j, h, :],
                                    pt[:, hh * CH + off:hh * CH + off + w],
                                    start=first, stop=(j == jplan[-1][0]))
                            first = False
                        for h in (h0, h1):
                            zdst = zt01 if h < 2 else zt23
                            zpo = (h % 2) * HD
                            recip = small.tile([1, CH], f32r, tag="recip",
                                               name="recip")
                            with nc.allow_low_precision(reason="fp22 recip"):
                                nc.vector.reciprocal(recip,
                                                     ztaus[h][HD:HD + 1, :])
                            bc = psum.tile([P, CH], f32, tag="ps512", bufs=4,
                                           name="bc")
                            nc.tensor.matmul(bc, ones_row, recip,
                                             start=True, stop=True)
                            bcs = small.tile([P, CH], f32, tag="bcs",
                                             name="bcs")
                            if h % 2 == 0:
                                nc.scalar.copy(bcs, bc)
                            else:
                                nc.vector.tensor_copy(bcs, bc)
                            nc.vector.tensor_mul(
                                zdst[zpo:zpo + HD, c * CH:(c + 1) * CH],
                                ztaus[h][0:HD, :], bcs[0:HD, :])
                    # out-proj for this chunk's 4 s-tiles
                    for st in range(4):
                        sg = c * 4 + st
                        ob = osb.tile([P, D], bf16, tag="ob", name="ob")
                        for nchunk in range(2):
                            ps = psum.tile([P, CH], f32, tag="ps512",
                                           bufs=4, name="ps")
                            for k, zsrc in enumerate((zt01, zt23)):
                                nc.tensor.matmul(
                                    ps, zsrc[:, sg * P:(sg + 1) * P],
                                    woT_sb[:, k, nchunk * CH:(nchunk + 1) * CH],
                                    start=(k == 0), stop=(k == 1))
                            if nchunk == 0:
                                nc.scalar.copy(
                                    ob[:, nchunk * CH:(nchunk + 1) * CH], ps)
                            else:
                                nc.vector.tensor_copy(
                                    ob[:, nchunk * CH:(nchunk + 1) * CH], ps)
                        nc.sync.dma_start(out[sg * P:(sg + 1) * P, :], ob)
            for pool_ in (osb, small, ptp, psum, wsb, xTp):
                pool_.release()
    nc.compile()
    # Belt-and-braces: any write-only preamble registers that survive DCE
    # but never get ids from alloc_regs would fail walrus birverifier
    # (reg_id == -1). They are write-only, so engine-unique ids are safe;
    # keep _lo/_hi pairs adjacent and even-aligned.
    from collections import defaultdict
    ctr = defaultdict(int)
    for f_ in nc.m.functions:
        for a in f_.allocations:
            if isinstance(a, mybir.Register) and a.reg_id >= 0:
                ctr[a.engine] = max(ctr[a.engine], a.reg_id + 1)
    for f_ in nc.m.functions:
        for a in f_.allocations:
            if isinstance(a, mybir.Register) and a.reg_id == -1:
                if a.name.endswith("_lo") and ctr[a.engine] % 2:
                    ctr[a.engine] += 1
                a.reg_id = ctr[a.engine]
                ctr[a.engine] += 1
    return nc


def kernel(query, key, value, mask, key_padding_mask,
           Wq, bq, Wk, bk, Wv, bv, Wo, bo, _return_perf=False):
    from concourse import bass_utils

    query = np.asarray(query, np.float32)
    key_ = np.asarray(key, np.float32)
    value = np.asarray(value, np.float32)
    Wq, Wk, Wv, Wo = (np.asarray(w, np.float32) for w in (Wq, Wk, Wv, Wo))
    bq, bk, bv, bo = (np.asarray(b_, np.float32) for b_ in (bq, bk, bv, bo))

    process, biased, bias_data = _block_structure(mask, key_padding_mask)
    bias_slots = {}
    for i in range(NT):
        for j in range(NT):
            if process[i, j] and biased[i, j]:
                bias_slots[(i, j)] = len(bias_slots)

    key_struct = (process.tobytes(), biased.tobytes())
    if key_struct not in _cache:
        _cache[key_struct] = _build_bass(process, biased, bias_slots)
    nc = _cache[key_struct]

    nbias = max(1, len(bias_slots))
    import ml_dtypes
    bf = ml_dtypes.bfloat16
    xT = {}
    for b in range(B):
        xT[("q", b)] = np.ascontiguousarray(query[b].T.astype(bf))
        xT[("k", b)] = np.ascontiguousarray(key_[b].T.astype(bf))
        xT[("v", b)] = np.ascontiguousarray(value[b].T.astype(bf))
    in_maps = []
    for core in range(8):
        b, g = core // G, core % G
        sl = slice(g * OG, (g + 1) * OG)
        bt = np.zeros((nbias, P, P), np.float32)
        for (i, j), slot in bias_slots.items():
            bt[slot] = bias_data[b][(i, j)]
        in_maps.append({
            "xqT": xT[("q", b)],
            "xkT": xT[("k", b)],
            "xvT": xT[("v", b)],
            "wqT": np.ascontiguousarray(Wq[sl].T.astype(bf)),
            "wkT": np.ascontiguousarray(Wk[sl].T.astype(bf)),
            "wvT": np.ascontiguousarray(Wv[sl].T.astype(bf)),
            "woT": np.ascontiguousarray(Wo[:, sl].T),
            "bq": np.ascontiguousarray(bq[sl]),
            "bk": np.ascontiguousarray(bk[sl]),
            "bv": np.ascontiguousarray(bv[sl]),
            "biasT": bt,
        })

    trace = bool(int(os.environ.get("KERNEL_TRACE", "0")))
    res = bass_utils.run_bass_kernel_spmd(
        nc, in_maps, core_ids=list(range(8)), trace=trace)

    out = np.zeros((B, S, D), np.float32)
    for core in range(8):
        out[core // G] += res.results[core]["out"].astype(np.float32)
    out += bo[None, None, :]
    if _return_perf:
        return out, res
    return out


# revision 42
# speedup vs baseline: 1.0097x; 1.0085x over previous
"""Trainium2 Bass kernel for MultiHeadAttention (B=2, S=2048, D=1024, H=16).

Sharding: 8 cores = 2 (batch) x 4 (head groups of 4 heads / 256 proj cols).
Each core computes attention for its batch + head group and a partial
output projection [S, D]; host sums the 4 partials per batch and adds bo.

Device pipeline per core (all matmuls in float32r = fp22, full PE rate):
  1. Project from host-pretransposed activations/weights:
       K.T[o,s], Q.T[o,s]  (lhsT = W.T, rhs = x.T)
       V[s,o]              (lhsT = x.T, rhs = W.T), ones-augmented per head
  2. Per sq-chunk c (512), per sk-tile j, per head h:
     S.T[sk,sq] = K.T_h^T Q.T_h (K=64; head pairs land on PE row groups
     0-63/64-127 so two heads run concurrently), additive -3e4 mask bias
     on partial blocks (block structure from the real mask, computed on
     host), exp (scale=1/8) -> P.T, PV (K=128) -> Z.T_aug (row 64 =
     softmax denominator); then reciprocal + K=1 matmul broadcast ->
     scale Z.T into SBUF.
  3. Out-proj per s-tile: O_partial[s, dout] = Z.T^T @ Wo_g.T, DMA out.
"""

import math
import os
import sys

import numpy as np

sys.path.insert(0, "/opt/trn_rl_repo")
sys.path.insert(0, "/opt/trn_rl_repo/concourse")

B, S, D, H = 2, 2048, 1024, 16
HD = D // H  # 64
G = 4  # head groups (cores per batch)
OG = D // G  # 256 proj cols per core
HPG = H // G  # 4 heads per core
P = 128
NT = S // P  # 16 s-tiles
CH = 512  # sq chunk width
NCH = S // CH  # 4 chunks
KT = D // P  # 8 contraction tiles for projections
NEG = -30000.0  # additive mask bias (pre-scale)

_cache = {}


def _block_structure(mask, key_padding_mask):
    """Classify each 128x128 block of the [S,S] score matrix per batch.

    Returns (process, biased, bias_data) where
      process[i,j]  : bool  -- any batch needs block (sq-tile i, sk-tile j)
      biased[i,j]   : bool  -- some processed batch needs a bias on (i,j)
      bias_data[b]  : {(i,j): [128,128] f32 bias (TRANSPOSED: [sk,sq])}
    """
    mask = np.asarray(mask)
    kpm = np.asarray(key_padding_mask)
    full = np.zeros((B, NT, NT), dtype=bool)
    partial = np.zeros((B, NT, NT), dtype=bool)
    blocks = {}
    for b in range(B):
        for i in range(NT):
            mrow = mask[i * P:(i + 1) * P]
            for j in range(NT):
                mb = mrow[:, j * P:(j + 1) * P] | kpm[b, None, j * P:(j + 1) * P]
                if mb.all():
                    full[b, i, j] = True
                elif mb.any():
                    partial[b, i, j] = True
                    blocks[(b, i, j)] = mb
                else:
                    blocks[(b, i, j)] = None
    process = (~full).any(axis=0)
    biased = process & (full | partial).any(axis=0)
    bias_data = []
    for b in range(B):
        d = {}
        for i in range(NT):
            for j in range(NT):
                if not (process[i, j] and biased[i, j]):
                    continue
                if full[b, i, j]:
                    d[(i, j)] = np.full((P, P), NEG, np.float32)
                elif partial[b, i, j]:
                    d[(i, j)] = (blocks[(b, i, j)].T * NEG).astype(np.float32)
                else:
                    d[(i, j)] = np.zeros((P, P), np.float32)
        bias_data.append(d)
    return process, biased, bias_data


def _build_bass(process, biased, bias_slots):
    """Trace the Tile kernel. bias_slots: {(i,j): slot} for biased blocks."""
    import concourse.bass as bass
    import concourse.tile as tile
    from concourse import bacc, mybir

    f32 = mybir.dt.float32
    f32r = mybir.dt.float32r
    bf16 = mybir.dt.bfloat16
    nc = bacc.Bacc("TRN2", target_bir_lowering=False, debug=False,
                   enable_asserts=False)

    # Host supplies activations and weights already transposed.
    xqT = nc.dram_tensor("xqT", [D, S], bf16, kind="ExternalInput").ap()
    xkT = nc.dram_tensor("xkT", [D, S], bf16, kind="ExternalInput").ap()
    xvT = nc.dram_tensor("xvT", [D, S], bf16, kind="ExternalInput").ap()
    wqT = nc.dram_tensor("wqT", [D, OG], bf16, kind="ExternalInput").ap()
    wkT = nc.dram_tensor("wkT", [D, OG], bf16, kind="ExternalInput").ap()
    wvT = nc.dram_tensor("wvT", [D, OG], bf16, kind="ExternalInput").ap()
    woT = nc.dram_tensor("woT", [OG, D], f32r, kind="ExternalInput").ap()
    bq = nc.dram_tensor("bq", [OG], f32, kind="ExternalInput").ap()
    bk = nc.dram_tensor("bk", [OG], f32, kind="ExternalInput").ap()
    bv = nc.dram_tensor("bv", [OG], f32, kind="ExternalInput").ap()
    nbias = max(1, len(bias_slots))
    biasT = nc.dram_tensor("biasT", [nbias, P, P], f32,
                           kind="ExternalInput").ap()
    out = nc.dram_tensor("out", [S, D], bf16, kind="ExternalOutput").ap()

    xqTr = xqT.rearrange("(t p) s -> p t s", p=P)
    xkTr = xkT.rearrange("(t p) s -> p t s", p=P)
    xvTr = xvT.rearrange("(t p) s -> p t s", p=P)

    with tile.TileContext(nc) as tc:
        with tc.tile_pool(name="persist", bufs=1) as persist, \
             tc.tile_pool(name="const", bufs=1) as const:
            # Persistent SBUF tensors
            qT = persist.tile([P, 2, S], bf16)       # [o-part, o-tile, s]
            kT = persist.tile([P, 2, S], bf16)
            vaug = persist.tile([P, NT, HPG, HD + 1], bf16)
            zt01 = persist.tile([P, S], f32r)        # heads 0,1 Z.T scaled
            zt23 = persist.tile([P, S], f32r)
            woT_sb = persist.tile([P, 2, D], f32r)
            bias_sb = persist.tile([P, nbias, P], f32)

            ones_row = const.tile([1, P], f32r)
            one_bits = 0x3F800000  # 1.0f
            nc.vector.memset(ones_row.bitcast(mybir.dt.uint32), one_bits)
            bqs = const.tile([P, 2], f32)
            bks = const.tile([P, 2], f32)
            bvb = const.tile([P, OG], f32)

            nc.sync.dma_start(bqs, bq.rearrange("(t p) -> p t", p=P))
            nc.sync.dma_start(bks, bk.rearrange("(t p) -> p t", p=P))
            # broadcast bv across partitions
            nc.sync.dma_start(
                bvb, bass.AP(tensor=bv.tensor, offset=bv.offset,
                             ap=[[0, P]] + list(bv.ap)))
            nc.vector.memset(vaug[:, :, :, HD:HD + 1].bitcast(mybir.dt.uint16),
                             0x3F80)  # 1.0 in bf16

            # ---- Flat pools for the whole kernel (avoid release stalls) ----
            xTp = tc.alloc_tile_pool(name="xT", bufs=3)
            wsb = tc.alloc_tile_pool(name="wsb", bufs=1)
            psum = tc.alloc_tile_pool(name="psum", bufs=1, space="PSUM")
            ptp = tc.alloc_tile_pool(name="pt", bufs=6)
            small = tc.alloc_tile_pool(name="small", bufs=4)
            osb = tc.alloc_tile_pool(name="osb", bufs=3)
            if True:
                wqT_sb = wsb.tile([P, KT, OG], bf16, tag="w")
                wkT_sb = wsb.tile([P, KT, OG], bf16, tag="w2")
                wvT_sb = wsb.tile([P, KT, OG], bf16, tag="w3")
                nc.sync.dma_start(wkT_sb, wkT.rearrange("(t p) o -> p t o", p=P))
                nc.sync.dma_start(wvT_sb, wvT.rearrange("(t p) o -> p t o", p=P))
                nc.sync.dma_start(wqT_sb, wqT.rearrange("(t p) o -> p t o", p=P))

                # Projections ordered so attention can start ASAP:
                # all K chunks, Q chunk 0, all V chunks, Q chunks 1-3.
                # Deferred constant loads (bias tiles, Wo) are emitted
                # mid-stream so they don't delay the first projections.
                plan = ([(0, c) for c in range(NCH)] + [(2, 0)]
                        + [(1, c) for c in range(NCH)]
                        + [(2, c) for c in range(1, NCH)])
                srcs = {0: (xkTr, wkT_sb), 1: (xvTr, wvT_sb),
                        2: (xqTr, wqT_sb)}
                for step, (which, c) in enumerate(plan):
                    if step == 5:
                        nc.sync.dma_start(bias_sb,
                                          biasT.rearrange("n p q -> p n q"))
                    elif step == 8:
                        nc.sync.dma_start(
                            woT_sb, woT.rearrange("(t p) d -> p t d", p=P))
                    if True:
                        xr, w_sb = srcs[which]
                        xTc = xTp.tile([P, KT, CH], bf16, tag="xT",
                                       name="xTc")
                        nc.sync.dma_start(xTc, xr[:, :, c * CH:(c + 1) * CH])
                        if which != 1:
                            # K.T / Q.T : out [o(2 tiles), s-chunk]
                            dst = kT if which == 0 else qT
                            bias_ap = bks if which == 0 else bqs
                            for ot in range(2):
                                ps = psum.tile([P, CH], f32, tag="ps512",
                                               bufs=4, name="ps")
                                for k in range(KT):
                                    nc.tensor.matmul(
                                        ps, w_sb[:, k, ot * P:(ot + 1) * P],
                                        xTc[:, k, :],
                                        start=(k == 0), stop=(k == KT - 1))
                                nc.vector.tensor_scalar_add(
                                    dst[:, ot, c * CH:(c + 1) * CH], ps,
                                    bias_ap[:, ot:ot + 1])
                        else:
                            # V: out [s-tile, o]; bias broadcast via DVE
                            for st in range(CH // P):
                                ps = psum.tile([P, OG], f32, tag="ps512",
                                               bufs=4, name="ps")
                                for k in range(KT):
                                    nc.tensor.matmul(
                                        ps, xTc[:, k, st * P:(st + 1) * P],
                                        w_sb[:, k, :],
                                        start=(k == 0), stop=(k == KT - 1))
                                nc.vector.tensor_add(
                                    vaug[:, c * 4 + st, :, 0:HD],
                                    ps.rearrange("p (h d) -> p h d", h=HPG),
                                    bvb.rearrange("p (h d) -> p h d", h=HPG))

            # ---- Attention + out-proj, per sq-chunk ----
            if True:
                for c in range(NCH):
                    tiles_i = list(range(c * 4, c * 4 + 4))
                    jplan = []
                    for j in range(NT):
                        ii = [i for i in tiles_i if process[i, j]]
                        if ii:
                            jplan.append((j, min(ii) - c * 4,
                                          max(ii) - c * 4 + 1))
                    for hp in range(2):  # head pairs (2*hp, 2*hp+1)
                        h0, h1 = 2 * hp, 2 * hp + 1
                        ot = hp
                        ztaus = {}
                        for h in (h0, h1):
                            zta = psum.tile([HD + 1, CH], f32,
                                            tag=f"zt{h % 2}", bufs=1,
                                            name=f"ztau{h % 2}")
                            ztaus[h] = zta
                        first = True
                        for j, lo, hi in jplan:
                            off, w = lo * P, (hi - lo) * P
                            # both heads' S.T in one [P, 2*CH] psum tile:
                            # h0 -> cols [0, CH), h1 -> cols [CH, 2CH);
                            # base partitions 0/64 put them on different
                            # PE row groups (concurrent matmuls).
                            st_ = psum.tile([P, 2 * CH], f32, tag="st",
                                            bufs=2, name="st_")
                            for hh, h in enumerate((h0, h1)):
                                po = (h % 2) * HD
                                nc.tensor.matmul(
                                    st_[:, hh * CH + off:hh * CH + off + w],
                                    kT[po:po + HD, ot, j * P:(j + 1) * P],
                                    qT[po:po + HD, ot,
                                       c * CH + off:c * CH + off + w],
                                    start=True, stop=True)
                            for i in range(c * 4 + lo, c * 4 + hi):
                                if biased[i, j]:
                                    sl = bias_slots[(i, j)]
                                    so = (i - c * 4) * P
                                    bap = bias_sb[:, sl, :]
                                    bcast2 = bass.AP(
                                        tensor=bap.tensor, offset=bap.offset,
                                        ap=[bap.ap[0], [0, 2]] + list(bap.ap[1:]))
                                    stv = st_[:, so:so + P]
                                    st2 = bass.AP(
                                        tensor=stv.tensor, offset=stv.offset,
                                        ap=[stv.ap[0], [CH, 2]] + list(stv.ap[1:]))
                                    nc.vector.tensor_add(st2, st2, bcast2)
                            pt = ptp.tile([P, 2 * CH], bf16, tag="pt",
                                          name="pt")
                            pt2 = pt.rearrange("p (a b) -> p a b", a=2)
                            stq = st_.rearrange("p (a b) -> p a b", a=2)
                            nc.scalar.activation(
                                pt2[:, :, off:off + w], stq[:, :, off:off + w],
                                mybir.ActivationFunctionType.Exp,
                                scale=1.0 / math.sqrt(HD))
                            for hh, h in enumerate((h0, h1)):
                                nc.tensor.matmul(
                                    ztaus[h][:, off:off + w],
                                    vaug[:, j, h, :],
                                    pt[:, hh * CH + off:hh * CH + off + w],
                                    start=first, stop=(j == jplan[-1][0]))
                            first = False
                        for h in (h0, h1):
                            zdst = zt01 if h < 2 else zt23
                            zpo = (h % 2) * HD
                            recip = small.tile([1, CH], f32r, tag="recip",
                                               name="recip")
                            with nc.allow_low_precision(reason="fp22 recip"):
                                nc.vector.reciprocal(recip,
                                                     ztaus[h][HD:HD + 1, :])
                            bc = psum.tile([P, CH], f32, tag="ps512", bufs=4,
                                           name="bc")
                            nc.tensor.matmul(bc, ones_row, recip,
                                             start=True, stop=True)
                            bcs = small.tile([P, CH], f32, tag="bcs",
                                             name="bcs")
                            if h % 2 == 0:
                                nc.scalar.copy(bcs, bc)
                            else:
                                nc.vector.tensor_copy(bcs, bc)
                            nc.vector.tensor_mul(
                                zdst[zpo:zpo + HD, c * CH:(c + 1) * CH],
                                ztaus[h][0:HD, :], bcs[0:HD, :])
                    # out-proj for this chunk's 4 s-tiles
                    for st in range(4):
                        sg = c * 4 + st
                        ob = osb.tile([P, D], bf16, tag="ob", name="ob")
                        for nchunk in range(2):
                            ps = psum.tile([P, CH], f32, tag="ps512",
                                           bufs=4, name="ps")
                            for k, zsrc in enumerate((zt01, zt23)):
                                nc.tensor.matmul(
                                    ps, zsrc[:, sg * P:(sg + 1) * P],
                                    woT_sb[:, k, nchunk * CH:(nchunk + 1) * CH],
                                    start=(k == 0), stop=(k == 1))
                            if nchunk == 0:
                                nc.scalar.copy(
                                    ob[:, nchunk * CH:(nchunk + 1) * CH], ps)
                            else:
                                nc.vector.tensor_copy(
                                    ob[:, nchunk * CH:(nchunk + 1) * CH], ps)
                        nc.sync.dma_start(out[sg * P:(sg + 1) * P, :], ob)
            for pool_ in (osb, small, ptp, psum, wsb, xTp):
                pool_.release()
    nc.compile()
    # Belt-and-braces: any write-only preamble registers that survive DCE
    # but never get ids from alloc_regs would fail walrus birverifier
    # (reg_id == -1). They are write-only, so engine-unique ids are safe;
    # keep _lo/_hi pairs adjacent and even-aligned.
    from collections import defaultdict
    ctr = defaultdict(int)
    for f_ in nc.m.functions:
        for a in f_.allocations:
            if isinstance(a, mybir.Register) and a.reg_id >= 0:
                ctr[a.engine] = max(ctr[a.engine], a.reg_id + 1)
    for f_ in nc.m.functions:
        for a in f_.allocations:
            if isinstance(a, mybir.Register) and a.reg_id == -1:
                if a.name.endswith("_lo") and ctr[a.engine] % 2:
                    ctr[a.engine] += 1
                a.reg_id = ctr[a.engine]
                ctr[a.engine] += 1
    return nc


def kernel(query, key, value, mask, key_padding_mask,
           Wq, bq, Wk, bk, Wv, bv, Wo, bo, _return_perf=False):
    from concourse import bass_utils

    query = np.asarray(query, np.float32)
    key_ = np.asarray(key, np.float32)
    value = np.asarray(value, np.float32)
    Wq, Wk, Wv, Wo = (np.asarray(w, np.float32) for w in (Wq, Wk, Wv, Wo))
    bq, bk, bv, bo = (np.asarray(b_, np.float32) for b_ in (bq, bk, bv, bo))

    process, biased, bias_data = _block_structure(mask, key_padding_mask)
    bias_slots = {}
    for i in range(NT):
        for j in range(NT):
            if process[i, j] and biased[i, j]:
                bias_slots[(i, j)] = len(bias_slots)

    key_struct = (process.tobytes(), biased.tobytes())
    if key_struct not in _cache:
        _cache[key_struct] = _build_bass(process, biased, bias_slots)
    nc = _cache[key_struct]

    nbias = max(1, len(bias_slots))
    import ml_dtypes
    bf = ml_dtypes.bfloat16
    xT = {}
    for b in range(B):
        xT[("q", b)] = np.ascontiguousarray(query[b].T.astype(bf))
        xT[("k", b)] = np.ascontiguousarray(key_[b].T.astype(bf))
        xT[("v", b)] = np.ascontiguousarray(value[b].T.astype(bf))
    in_maps = []
    for core in range(8):
        b, g = core // G, core % G
        sl = slice(g * OG, (g + 1) * OG)
        bt = np.zeros((nbias, P, P), np.float32)
        for (i, j), slot in bias_slots.items():
            bt[slot] = bias_data[b][(i, j)]
        in_maps.append({
            "xqT": xT[("q", b)],
            "xkT": xT[("k", b)],
            "xvT": xT[("v", b)],
            "wqT": np.ascontiguousarray(Wq[sl].T.astype(bf)),
            "wkT": np.ascontiguousarray(Wk[sl].T.astype(bf)),
            "wvT": np.ascontiguousarray(Wv[sl].T.astype(bf)),
            "woT": np.ascontiguousarray(Wo[:, sl].T),
            "bq": np.ascontiguousarray(bq[sl]),
            "bk": np.ascontiguousarray(bk[sl]),
            "bv": np.ascontiguousarray(bv[sl]),
            "biasT": bt,
        })

    trace = bool(int(os.environ.get("KERNEL_TRACE", "0")))
    res = bass_utils.run_bass_kernel_spmd(
        nc, in_maps, core_ids=list(range(8)), trace=trace)

    out = np.zeros((B, S, D), np.float32)
    for core in range(8):
        out[core // G] += res.results[core]["out"].astype(np.float32)
    out += bo[None, None, :]
    if _return_perf:
        return out, res
    return out
